# revision 1
# baseline (speedup 1.0000x reference)
"""Trainium2 Bass kernel for AdpHeadedAttention.

Strategy: data-parallel over (batch, T1-half) -> 8 cores, one full
multi-head attention per core on 512 query rows. All rescales
(a_q_f / norm_qq / sqrt(DK), 1 / norm_kk) are folded into the Q/K
projection weights on the host; compute runs in bf16 on the
TensorEngine with fp32 PSUM accumulation.

Layout trick: projections for q/k produce transposed activations
[chan, rows] so attention needs no on-chip transposes:
  scoresT[k,q] = kT_head.T @ qT_head          (K=dk=64)
  expT = Exp(scoresT + mask_bias_per_partition)
  xT[dv,q]    = v_aug.T @ expT                (K=128, M=65: row 64 = denom)
  out[r,c]    = xT.T @ WoT  (+ bias via ones-row matmul)
"""

import numpy as np
import ml_dtypes

B, T1, T2, F, H = 4, 1024, 1024, 1024, 16
DK = F // H  # 64
N_CORES = 8
RPC = 512  # query rows per core

BF16 = ml_dtypes.bfloat16


def build_nc(loop_k: int = 1):
    """Build the per-core Bass graph (identical on all 8 cores).

    loop_k > 1 wraps the whole compute body in a hardware For_i loop —
    used only for slope-based timing in test harnesses.
    """
    from contextlib import ExitStack

    import concourse.bacc as bacc
    import concourse.tile as tile
    from concourse import mybir

    BF = mybir.dt.bfloat16
    F32 = mybir.dt.float32
    AF = mybir.ActivationFunctionType

    nc = bacc.Bacc(None, target_bir_lowering=False, num_devices=N_CORES)

    qTin_d = nc.dram_tensor("qT_in", [F, RPC], BF, kind="ExternalInput")
    kTin_d = nc.dram_tensor("kT_in", [F, T2], BF, kind="ExternalInput")
    vTin_d = nc.dram_tensor("vT_in", [F, T2], BF, kind="ExternalInput")
    wqT_d = nc.dram_tensor("wqT", [F, F], BF, kind="ExternalInput")
    wkT_d = nc.dram_tensor("wkT", [F, F], BF, kind="ExternalInput")
    wvT_d = nc.dram_tensor("wvT", [F, F], BF, kind="ExternalInput")
    woT_d = nc.dram_tensor("woT", [F, F], BF, kind="ExternalInput")
    bq_d = nc.dram_tensor("bq", [F, 1], F32, kind="ExternalInput")
    bk_d = nc.dram_tensor("bk", [F, 1], F32, kind="ExternalInput")
    bv_d = nc.dram_tensor("bv", [1, F], BF, kind="ExternalInput")
    bo_d = nc.dram_tensor("bo", [1, F], BF, kind="ExternalInput")
    mb_d = nc.dram_tensor("maskb", [T2, 1], F32, kind="ExternalInput")
    out_d = nc.dram_tensor("out", [RPC, F], F32, kind="ExternalOutput")

    with tile.TileContext(nc) as tc, ExitStack() as ctx:
        const = ctx.enter_context(tc.tile_pool(name="const", bufs=1))

        def load3(dram, cols, name):
            t = const.tile([128, 8, cols], BF, tag=name)
            nc.sync.dma_start(out=t[:], in_=dram.ap().rearrange("(a p) c -> p a c", p=128))
            return t

        wq_sb = load3(wqT_d, F, "wq")
        wk_sb = load3(wkT_d, F, "wk")
        wv_sb = load3(wvT_d, F, "wv")
        wo_sb = load3(woT_d, F, "wo")
        qTin_sb = load3(qTin_d, RPC, "qTin")
        kTin_sb = load3(kTin_d, T2, "kTin")
        vTin_sb = load3(vTin_d, T2, "vTin")

        def loadbias(dram, name):
            t = const.tile([128, 8, 1], F32, tag=name)
            nc.sync.dma_start(out=t[:], in_=dram.ap().rearrange("(a p) o -> p a o", p=128))
            return t

        bq_sb = loadbias(bq_d, "bq")
        bk_sb = loadbias(bk_d, "bk")
        mb_sb = loadbias(mb_d, "mb")

        bv_sb = const.tile([1, F], BF, tag="bv")
        nc.sync.dma_start(out=bv_sb[:], in_=bv_d.ap())
        bo_sb = const.tile([1, F], BF, tag="bo")
        nc.sync.dma_start(out=bo_sb[:], in_=bo_d.ap())

        ones_bf = const.tile([1, 128], BF, tag="ones_bf")
        nc.vector.memset(ones_bf[:], 1.0)
        ones_f = const.tile([1, 64], F32, tag="ones_f")
        nc.vector.memset(ones_f[:], 1.0)

        acts = ctx.enter_context(tc.tile_pool(name="acts", bufs=1))
        expp = ctx.enter_context(tc.tile_pool(name="expp", bufs=2))
        rbp = ctx.enter_context(tc.tile_pool(name="rbp", bufs=2))
        outp = ctx.enter_context(tc.tile_pool(name="outp", bufs=2))
        psA = ctx.enter_context(tc.tile_pool(name="psA", bufs=2, space="PSUM"))
        psS = ctx.enter_context(tc.tile_pool(name="psS", bufs=2, space="PSUM"))
        psX = ctx.enter_context(tc.tile_pool(name="psX", bufs=2, space="PSUM"))
        psR = ctx.enter_context(tc.tile_pool(name="psR", bufs=2, space="PSUM"))

        def body(_it, unroll=1):
            qT_sb = acts.tile([128, 8, RPC], BF, tag="qT")
            kT_sb = acts.tile([128, 8, T2], BF, tag="kT")
            v_sb = acts.tile([128, 8, H, DK + 1], BF, tag="v")
            xT_sb = acts.tile([128, 8, RPC], BF, tag="xT")
            nc.vector.memset(v_sb[:, :, :, DK : DK + 1], 1.0)

            # ---- qT = (Wq_s @ queryT) + bq_s : [chan, rows] ----
            for cc in range(8):
                ps = psA.tile([128, 512], F32, tag="A")
                for fi in range(8):
                    nc.tensor.matmul(
                        ps[:],
                        wq_sb[:, fi, cc * 128 : (cc + 1) * 128],
                        qTin_sb[:, fi, :],
                        start=(fi == 0),
                        stop=(fi == 7),
                    )
                nc.vector.tensor_scalar_add(qT_sb[:, cc, :], ps[:], bq_sb[:, cc, 0:1])

            # ---- kT = (Wk_s @ keyT) + bk_s : [chan, rows] ----
            for cc in range(8):
                for hf in range(2):
                    ps = psA.tile([128, 512], F32, tag="A")
                    for fi in range(8):
                        nc.tensor.matmul(
                            ps[:],
                            wk_sb[:, fi, cc * 128 : (cc + 1) * 128],
                            kTin_sb[:, fi, hf * 512 : (hf + 1) * 512],
                            start=(fi == 0),
                            stop=(fi == 7),
                        )
                    nc.vector.tensor_scalar_add(
                        kT_sb[:, cc, hf * 512 : (hf + 1) * 512], ps[:], bk_sb[:, cc, 0:1]
                    )

            # ---- v = value @ Wv.T + bv : [rows, chan], chan packed as (h, dk+1) ----
            for rt in range(8):
                for hf in range(2):
                    ps = psA.tile([128, 512], F32, tag="A")
                    for fi in range(8):
                        nc.tensor.matmul(
                            ps[:],
                            vTin_sb[:, fi, rt * 128 : (rt + 1) * 128],
                            wv_sb[:, fi, hf * 512 : (hf + 1) * 512],
                            start=(fi == 0),
                            stop=False,
                        )
                    nc.tensor.matmul(
                        ps[:],
                        ones_bf[0:1, :],
                        bv_sb[0:1, hf * 512 : (hf + 1) * 512],
                        start=False,
                        stop=True,
                    )
                    nc.vector.tensor_copy(
                        v_sb[:, rt, hf * 8 : (hf + 1) * 8, 0:DK],
                        ps[:].rearrange("p (a b) -> p a b", a=8),
                    )

            # ---- attention per head ----
            for h in range(H):
                cc, po = h // 2, (h % 2) * 64
                expT = expp.tile([128, 8, 512], BF, tag="expT")
                for kc in range(8):
                    pss = psS.tile([128, 512], F32, tag="S")
                    nc.tensor.matmul(
                        pss[:],
                        kT_sb[po : po + 64, cc, kc * 128 : (kc + 1) * 128],
                        qT_sb[po : po + 64, cc, :],
                        start=True,
                        stop=True,
                    )
                    nc.scalar.activation(
                        expT[:, kc, :], pss[:], AF.Exp, bias=mb_sb[:, kc, 0:1], scale=1.0
                    )
                psx = psX.tile([65, 512], F32, tag="X")
                for kc in range(8):
                    nc.tensor.matmul(
                        psx[:],
                        v_sb[:, kc, h, :],
                        expT[:, kc, :],
                        start=(kc == 0),
                        stop=(kc == 7),
                    )
                recip = rbp.tile([1, 512], F32, tag="recip")
                nc.vector.reciprocal(recip[:], psx[64:65, :])
                psr = psR.tile([64, 512], F32, tag="R")
                nc.tensor.matmul(psr[:], ones_f[0:1, :], recip[:], start=True, stop=True)
                rbc = rbp.tile([64, 512], F32, tag="rbc")
                nc.scalar.copy(rbc[:], psr[:])
                nc.vector.tensor_mul(xT_sb[po : po + 64, cc, :], psx[0:64, :], rbc[:])

            # ---- out = x @ Wo.T + bo : [rows, chan] ----
            for rc in range(4):
                for hf in range(2):
                    ps = psA.tile([128, 512], F32, tag="A")
                    for cc in range(8):
                        nc.tensor.matmul(
                            ps[:],
                            xT_sb[:, cc, rc * 128 : (rc + 1) * 128],
                            wo_sb[:, cc, hf * 512 : (hf + 1) * 512],
                            start=(cc == 0),
                            stop=False,
                        )
                    nc.tensor.matmul(
                        ps[:],
                        ones_bf[0:1, :],
                        bo_sb[0:1, hf * 512 : (hf + 1) * 512],
                        start=False,
                        stop=True,
                    )
                    ot = outp.tile([128, 512], F32, tag="out")
                    nc.vector.tensor_copy(ot[:], ps[:])
                    nc.sync.dma_start(
                        out=out_d[rc * 128 : (rc + 1) * 128, hf * 512 : (hf + 1) * 512],
                        in_=ot[:],
                    )

        if loop_k == 1:
            body(0)
        else:
            with tc.For_i(0, loop_k, 1) as it:
                body(it)

    nc.finalize()
    return nc


def prepare_host(query, key, value, mask, Wq, bq, Wk, bk, Wv, bv, Wo, bo, a_q):
    """Host-side folding + sharding. Returns (in_maps, l_qk, a_q_f)."""
    query = np.asarray(query, np.float32)
    key = np.asarray(key, np.float32)
    value = np.asarray(value, np.float32)
    mask = np.asarray(mask)
    Wq = np.asarray(Wq, np.float32)
    bq = np.asarray(bq, np.float32)
    Wk = np.asarray(Wk, np.float32)
    bk = np.asarray(bk, np.float32)
    Wv = np.asarray(Wv, np.float32)
    bv = np.asarray(bv, np.float32)
    Wo = np.asarray(Wo, np.float32)
    bo = np.asarray(bo, np.float32)
    a_q = np.asarray(a_q, np.float32)

    norm_qq = np.linalg.norm(Wq, axis=1).astype(np.float32)
    norm_kk = np.linalg.norm(Wk, axis=1).astype(np.float32)
    a_q_max = np.max(np.abs(a_q))
    a_q_f = (a_q * (np.abs(a_q) > 0.001 * a_q_max)).astype(np.float32)
    l_qk = np.float32(np.sum(np.abs(a_q_f)))

    s_q = (a_q_f / norm_qq / np.sqrt(np.float32(DK))).astype(np.float32)
    s_k = (1.0 / norm_kk).astype(np.float32)

    wqT = np.ascontiguousarray((Wq * s_q[:, None]).T).astype(BF16)
    wkT = np.ascontiguousarray((Wk * s_k[:, None]).T).astype(BF16)
    wvT = np.ascontiguousarray(Wv.T).astype(BF16)
    woT = np.ascontiguousarray(Wo.T).astype(BF16)
    bq_s = (bq * s_q)[:, None].astype(np.float32)
    bk_s = (bk * s_k)[:, None].astype(np.float32)
    bv_r = np.ascontiguousarray(bv[None, :]).astype(BF16)
    bo_r = np.ascontiguousarray(bo[None, :]).astype(BF16)

    in_maps = []
    for i in range(N_CORES):
        b, half = divmod(i, 2)
        qT = np.ascontiguousarray(query[b, half * RPC : (half + 1) * RPC, :].T).astype(BF16)
        kT = np.ascontiguousarray(key[b].T).astype(BF16)
        vT = np.ascontiguousarray(value[b].T).astype(BF16)
        maskb = np.where(mask[b, 0] != 0, np.float32(0.0), np.float32(-50.0))[:, None]
        maskb = np.ascontiguousarray(maskb, np.float32)
        in_maps.append(
            {
                "qT_in": qT,
                "kT_in": kT,
                "vT_in": vT,
                "wqT": wqT,
                "wkT": wkT,
                "wvT": wvT,
                "woT": woT,
                "bq": bq_s,
                "bk": bk_s,
                "bv": bv_r,
                "bo": bo_r,
                "maskb": maskb,
            }
        )
    return in_maps, l_qk, a_q_f


def assemble_out(per_core_outs):
    out = np.empty((B, T1, F), np.float32)
    for i in range(N_CORES):
        b, half = divmod(i, 2)
        out[b, half * RPC : (half + 1) * RPC, :] = per_core_outs[i]
    return out


def kernel(query, key, value, mask, Wq, bq, Wk, bk, Wv, bv, Wo, bo, a_q):
    from concourse.bass_utils import run_bass_kernel_spmd

    in_maps, l_qk, a_q_f = prepare_host(
        query, key, value, mask, Wq, bq, Wk, bk, Wv, bv, Wo, bo, a_q
    )
    nc = build_nc(loop_k=1)
    res = run_bass_kernel_spmd(nc, in_maps, core_ids=list(range(N_CORES)))
    out = assemble_out([res.results[i]["out"] for i in range(N_CORES)])
    return out, l_qk, a_q_f
